# revision 6
# baseline (speedup 1.0000x reference)
"""Trainium2 Bass kernel for BaseLayerWithLoRA.

Computes out = x @ W.T + bias + (x @ A.T) @ B.T for
x [2, 4096, 4096], W [4096, 4096], bias [4096], A [16, 4096], B [4096, 16].

Strategy
--------
The LoRA path is rank-16, so it folds into the base weight on the host:

    W' = W + B @ A        (fp32, host)
    out = x @ W'.T + bias (device: one GEMM + per-partition bias add)

Device math runs in bf16 (fp32 PSUM accumulation): rel-err ~2.3e-3,
well inside the 2e-2 gate, and it halves HBM traffic vs fp32.

Sharding: data-parallel over tokens (8192 -> 1024/core). Each core keeps
its x.T shard (8 MiB bf16) resident in SBUF and streams W' (32 MiB bf16)
exactly once.

Layout: W' tiles are the stationary operand ([128 d_in, 128 d_out]), x.T
tiles the moving operand ([128 d_in, 512 tokens]) -> PSUM [128 d_out,
512 tok]; the output is produced transposed (outT [d_out, tok]) and
un-transposed on the host. Panels are processed in groups of 4,
t-interleaved, so all 8 PSUM banks accumulate concurrently and the PE
never waits on the x-shard DMA during ramp-up. Bias is added during
PSUM eviction (scalar engine for even banks, vector for odd) and the
result is stored via the sync HWDGE queue.
"""

import os
import sys

for _p in ("/opt/trn_rl_repo", "/opt/pypackages"):
    if _p not in sys.path:
        sys.path.append(_p)

# The kernel executes on the axon-tunneled NeuronCores via PJRT; a
# JAX_PLATFORMS=cpu pin (used by some reference harnesses) would hide them.
_jp = os.environ.get("JAX_PLATFORMS")
if _jp and "axon" not in _jp:
    del os.environ["JAX_PLATFORMS"]

import numpy as np
import concourse.bacc as bacc
import concourse.mybir as mybir
from concourse.tile import TileContext
from concourse.bass_utils import run_bass_kernel_spmd

F32 = mybir.dt.float32
BF16 = mybir.dt.bfloat16
NP_BF16 = mybir.dt.np(BF16)

BATCH, SEQ, D_IN, D_OUT, RANK = 2, 4096, 4096, 4096, 16
N_CORES = 8
TOK = BATCH * SEQ            # 8192 tokens total
TOK_C = TOK // N_CORES       # 1024 tokens per core
P = 128                      # partitions
NT = D_IN // P               # 32 contraction (k) tiles
NPO = D_OUT // P             # 32 output panels of 128 features
H = TOK_C // 512             # 2 moving-operand blocks of 512 tokens
GRP = 4                      # panels interleaved t-major per group
NG = NPO // GRP              # 8 groups

_NC_CACHE = None


def _build_nc():
    """Trace + schedule + compile the per-core Bass module (SPMD: all 8
    cores run this same program on their own token shard)."""
    nc = bacc.Bacc(None, target_bir_lowering=False, debug=False)

    xT = nc.dram_tensor("xT", [D_IN, TOK_C], BF16, kind="ExternalInput")
    # W' pre-packed on host: Wp[p, (po*NT + t)*P + m] = W'[po*P+m, t*P+p]
    # so each 128-feature panel is one contiguous [128, NT*P] slab.
    Wp = nc.dram_tensor("Wp", [P, NPO * NT * P], BF16, kind="ExternalInput")
    biasT = nc.dram_tensor("biasT", [P, NPO], F32, kind="ExternalInput")
    outT = nc.dram_tensor("outT", [D_OUT, TOK_C], F32, kind="ExternalOutput")

    xT_t = xT.rearrange("(t p) n -> t p n", p=P)

    with TileContext(nc) as tc:
        with (
            tc.tile_pool(name="xpool", bufs=1) as xpool,
            tc.tile_pool(name="cpool", bufs=1) as cpool,
            tc.tile_pool(name="wpool", bufs=32) as wpool,
            tc.tile_pool(name="opool", bufs=8) as opool,
            tc.tile_pool(name="pspool", bufs=1, space="PSUM") as pspool,
        ):
            # Bias first on the scalar HWDGE ring (tiny, ahead of W').
            bias_sb = cpool.tile([P, NPO], F32, name="bias_sb", tag="bias_sb")
            nc.scalar.dma_start(out=bias_sb[:], in_=biasT[:])

            # W' panels stream on the scalar ring in quarter-panel chunks
            # (the first matmul only waits on 256 KiB, not a full panel);
            # 32 rotating buffers give ~2 groups of prefetch depth.
            CH = NT // 4  # k-tiles per chunk
            wch = []
            for po in range(NPO):
                for c in range(4):
                    wt = wpool.tile([P, CH * P], BF16, name=f"wt{po}_{c}", tag="wt")
                    base = po * NT * P + c * CH * P
                    nc.scalar.dma_start(out=wt[:], in_=Wp[:, base : base + CH * P])
                    wch.append(wt)

            # Resident x.T shard on the sync ring: 32 tiles [128, 1024].
            xts = []
            for t in range(NT):
                xt = xpool.tile([P, TOK_C], BF16, name=f"xt{t}", tag=f"xt{t}")
                nc.sync.dma_start(out=xt[:], in_=xT_t[t])
                xts.append(xt)

            # 8 PSUM banks: groups of 4 panels x 2 token blocks accumulate
            # concurrently (t-major), so the PE keeps up with the x DMA
            # from the first tile onward.
            for g in range(NG):
                psums = {}
                for j in range(GRP):
                    for h in range(H):
                        b = j * H + h
                        psums[b] = pspool.tile(
                            [P, 512], F32, name=f"ps_{g}_{b}", tag=f"ps{b}"
                        )
                for t in range(NT):
                    for j in range(GRP):
                        po = g * GRP + j
                        w = wch[po * 4 + t // CH][:, (t % CH) * P : (t % CH + 1) * P]
                        for h in range(H):
                            nc.tensor.matmul(
                                psums[j * H + h][:],
                                w,
                                xts[t][:, h * 512 : (h + 1) * 512],
                                start=(t == 0),
                                stop=(t == NT - 1),
                            )
                for j in range(GRP):
                    po = g * GRP + j
                    for h in range(H):
                        b = j * H + h
                        ot = opool.tile([P, 512], F32, name=f"ot_{g}_{b}", tag="ot")
                        # Bias-add during eviction; split banks across the
                        # scalar and vector engines so they drain in parallel.
                        if h == 0:
                            nc.scalar.add(ot[:], psums[b][:], bias_sb[:, po : po + 1])
                        else:
                            nc.vector.tensor_scalar_add(
                                ot[:], psums[b][:], bias_sb[:, po : po + 1]
                            )
                        # Alternate store rings so the final group's stores
                        # drain on two HWDGE rings (halves the tail).
                        ring = nc.sync if h == 0 else nc.scalar
                        ring.dma_start(
                            out=outT[po * P : (po + 1) * P, h * 512 : (h + 1) * 512],
                            in_=ot[:],
                        )

    nc.compile()
    return nc


def _get_nc():
    global _NC_CACHE
    if _NC_CACHE is None:
        _NC_CACHE = _build_nc()
    return _NC_CACHE


def _prep_inputs(x, W, bias, A, B):
    """Host-side fold + layout prep + sharding. Returns per-core inputs."""
    x_flat = np.asarray(x, dtype=np.float32).reshape(TOK, D_IN)
    Wf = np.asarray(W, dtype=np.float32) + np.asarray(B, dtype=np.float32) @ np.asarray(
        A, dtype=np.float32
    )
    # Pack W'.T into per-panel stationary-tile slabs (see _build_nc).
    # Wp[p, ((po*NT)+t)*P+m] = W'T[t*P+p, po*P+m]
    Wp = np.ascontiguousarray(
        np.ascontiguousarray(Wf.T)
        .reshape(NT, P, NPO, P)
        .transpose(1, 2, 0, 3)
        .reshape(P, NPO * NT * P)
        .astype(NP_BF16)
    )
    biasT = np.ascontiguousarray(
        np.asarray(bias, dtype=np.float32).reshape(NPO, P).T
    )
    x_bf = x_flat.astype(NP_BF16)
    in_maps = []
    for c in range(N_CORES):
        xT_c = np.ascontiguousarray(x_bf[c * TOK_C : (c + 1) * TOK_C, :].T)
        in_maps.append({"xT": xT_c, "Wp": Wp, "biasT": biasT})
    return in_maps


def _run(inputs, trace=False, trace_cores=None):
    nc = _get_nc()
    in_maps = _prep_inputs(**inputs)
    res = run_bass_kernel_spmd(
        nc,
        in_maps,
        core_ids=list(range(N_CORES)),
        trace=trace,
        trace_cores=trace_cores,
    )
    full = np.empty((TOK, D_OUT), dtype=np.float32)
    for c in range(N_CORES):
        full[c * TOK_C : (c + 1) * TOK_C, :] = res.results[c]["outT"].T
    return full.reshape(BATCH, SEQ, D_OUT), res


def kernel(**inputs):
    full, _ = _run(inputs, trace=False)
    return full


if __name__ == "__main__":
    rng = np.random.default_rng(0)
    inputs = {
        "x": rng.standard_normal((BATCH, SEQ, D_IN), dtype=np.float32),
        "W": rng.standard_normal((D_OUT, D_IN), dtype=np.float32) * 0.02,
        "bias": rng.standard_normal((D_OUT,), dtype=np.float32) * 0.02,
        "A": rng.standard_normal((RANK, D_IN), dtype=np.float32) * 0.02,
        "B": rng.standard_normal((D_OUT, RANK), dtype=np.float32) * 0.02,
    }
    got = kernel(**inputs)
    x64 = inputs["x"].reshape(TOK, D_IN).astype(np.float64)
    exp = x64 @ inputs["W"].astype(np.float64).T + inputs["bias"]
    exp += (x64 @ inputs["A"].astype(np.float64).T) @ inputs["B"].astype(np.float64).T
    exp = exp.reshape(BATCH, SEQ, D_OUT)
    rel = np.linalg.norm(got - exp) / np.linalg.norm(exp)
    print("self-check relative error:", rel)
